# revision 8
# baseline (speedup 1.0000x reference)
"""Causal flash attention for Trainium2, sharded 2 heads/core over 8 cores.

Math per head: out = softmax_causal(Q K^T / sqrt(D)) @ V,  Q/K/V [S=2048, D=64] fp32.

Device layout (per core, heads h0=2c, h1=2c+1):
  qT   [128, 2048]  rows 64h+d = Q[h]^T        (D on partitions, both heads stacked)
  kT   [128, 2048]  same for K
  vaug [2, 128, 1040] vaug[h, p, 65*kc+d] = V[h, 128*kc+p, d], d=64 column is ones
  outT [2, 64, 2048]  out[h]^T (normalized)

Scores are computed transposed (S^T[k, q] = K_chunk @ Q^T) so no transposes are
needed anywhere: softmax denominator comes out of the PV matmul via the ones
column of vaug (psum row 64), and the final division broadcasts 1/denom across
partitions with gpsimd.partition_broadcast.

The kernel is paced by the Scalar (ACT) engine's exp over every causal score
(34816 columns/core at 1 elem/cycle/lane, 1.2 GHz). Structure keeps ACT
saturated at that floor:
  - scores land in psum [128,1024] tiles (h0 in bank A, h1 in bank B; the
    head-pair matmuls run concurrently on PE row groups 0-63/64-127), one exp
    instruction per tile covering both heads.
  - diagonal chunks (width w < 512) are END-aligned against the bank boundary
    (h0 at [512-w:512], h1 at [512:512+w]) so the exp covers 2w contiguous
    columns with zero gap waste; the two smallest diagonal chunks of each span
    share one tile, giving 36 ACTIVATEs and exactly S^2/2-worth of columns.
  - causal masking is folded into the PE: an extra accumulating matmul adds a
    precomputed -480 strictly-lower-triangular bias (exp(0.125*x - 60) == 0)
    into the diagonal psum blocks before the exp, so the DVE carries no mask
    work and the exp->PV chain has no cross-engine mask hop.
  - the PE is prewarmed with dummy matmuls during the initial DMA wait so the
    HAM throttle ramps to full speed (2.4 GHz) before the first real matmul
    instead of ~6us into the kernel.
  - inputs arrive as ten DMAs on the sync/vector/gpsimd queues, smallest and
    first-needed pieces first (q span 0 + k chunk 0 gate the first matmul);
    the Scalar queue carries nothing but the exps.
  - each span's normalization tail is emitted one span late so every tail
    except the last hides under the next span's exp stream; the last tail is
    pipelined in column halves with its denominator copies on the then-idle
    scalar engine and the four stages interleaved across both heads.

HW-verified pitfalls honored here: custom DVE ops (reciprocal_approx_fast)
mis-read operands at nonzero partition offsets and from PSUM (denominator row
is first copied to a partition-0 SBUF tile); matmul PSUM output must be fp32
on TRN2; DVE instructions may read at most one PSUM operand; PE-write and
engine-read of the same PSUM bank is fatal (tails only read po after its
accumulation group closes).
"""

import os
import sys

import ml_dtypes
import numpy as np

sys.path.insert(0, "/opt/trn_rl_repo")

import concourse.bass as bass
import concourse.bacc as bacc
import concourse.mybir as mybir
import concourse.tile as tile
from concourse.bass_utils import run_bass_kernel_spmd

B, H, S, D = 1, 16, 2048, 64
N_CORES = 8
HEADS_PER_CORE = H // N_CORES  # 2
N_CHUNKS = S // 128  # 16 key chunks per head
N_SPANS = S // 512  # 4 query spans per head
F32 = mybir.dt.float32
BF16 = mybir.dt.bfloat16
F16 = mybir.dt.float16

PREWARM_MMS = 14  # dummy matmuls to ramp the PE while the first DMAs land
MASK_BIAS = -480.0  # pre-exp additive mask; exp(0.125 * (x - 480)) flushes to 0

_NC = None
_LAST_RESULTS = None


def _build_bass():
    nc = bacc.Bacc("TRN2", target_bir_lowering=False)
    qT = nc.declare_dram_parameter("qT", [128, S], F16, isOutput=False)
    kT = nc.declare_dram_parameter("kT", [128, S], F16, isOutput=False)
    vaug = nc.declare_dram_parameter("vaug", [2, 128, 65 * N_CHUNKS], BF16, isOutput=False)
    outT = nc.declare_dram_parameter("outT", [2, 64, S], F16, isOutput=True)

    with tile.TileContext(nc) as tc:
        with (
            tc.tile_pool(name="const", bufs=1) as const,
            tc.tile_pool(name="inbuf", bufs=1) as inbuf,
            tc.tile_pool(name="pbuf", bufs=6) as pbuf,
            tc.tile_pool(name="nbuf", bufs=2) as nbuf,
            tc.tile_pool(name="ps_s", bufs=2, space="PSUM") as ps_s,
            tc.tile_pool(name="ps_o", bufs=2, space="PSUM") as ps_o,
        ):
            # Prewarm source: gpsimd's very first op so the PE can start
            # ramping immediately (before gpsimd turns to its DMA issues).
            junk = const.tile([64, 128], BF16, tag="junk", name="junk")
            nc.gpsimd.memset(junk, 0.0)

            # Input loads. Scalar stays clean (it is the pacing engine); only
            # the sync and gpsimd queues can issue DMAs besides it. Pieces are
            # ordered by first-need time: q0 + k chunks 0-3 gate span 0.
            q0 = inbuf.tile([128, 512], F16, tag="q0", name="q0")
            nc.sync.dma_start(out=q0, in_=qT[:, 0:512])
            kx = inbuf.tile([128, 2048], F16, tag="kx", name="kx")
            nc.sync.dma_start(out=kx[:, 0:512], in_=kT[:, 0:512])
            v0 = inbuf.tile([128, 65 * N_CHUNKS], BF16, tag="v0", name="v0")
            nc.sync.dma_start(out=v0[:, 0:260], in_=vaug[0][:, 0:260])
            q1 = inbuf.tile([128, 512], F16, tag="q1", name="q1")
            nc.sync.dma_start(out=q1, in_=qT[:, 512:1024])
            nc.sync.dma_start(out=v0[:, 260:1040], in_=vaug[0][:, 260:1040])
            q23 = inbuf.tile([128, 1024], F16, tag="q23", name="q23")
            nc.sync.dma_start(out=q23, in_=qT[:, 1024:2048])

            v1 = inbuf.tile([128, 65 * N_CHUNKS], BF16, tag="v1", name="v1")
            nc.gpsimd.dma_start(out=v1[:, 0:260], in_=vaug[1][:, 0:260])
            nc.gpsimd.dma_start(out=kx[:, 512:1024], in_=kT[:, 512:1024])
            nc.gpsimd.dma_start(out=v1[:, 260:1040], in_=vaug[1][:, 260:1040])
            nc.gpsimd.dma_start(out=kx[:, 1024:2048], in_=kT[:, 1024:2048])
            vsb = [v0, v1]

            # Diagonal-mask constants (needed first at ~1 tile into span 0).
            ident = const.tile([128, 128], BF16, tag="ident", name="ident")
            nc.gpsimd.memset(ident, 1.0)
            nc.gpsimd.affine_select(
                out=ident,
                in_=ident,
                compare_op=mybir.AluOpType.is_equal,
                fill=0.0,
                base=0,
                pattern=[[1, 128]],
                channel_multiplier=-1,
            )
            # btri[p, q] = MASK_BIAS where q < p (strict lower triangle),
            # 0 elsewhere -- including all of cols 128:512, so one [128, w]
            # slice masks a diagonal block of any width.
            btri = const.tile([128, 512], BF16, tag="btri", name="btri")
            nc.gpsimd.memset(btri, MASK_BIAS)
            nc.gpsimd.affine_select(
                out=btri,
                in_=btri,
                compare_op=mybir.AluOpType.is_ge,
                fill=0.0,
                base=-1,
                pattern=[[-1, 512]],
                channel_multiplier=1,
            )

            def k_slice(h, kc):
                # kT chunk [64, 128] for head h: lhsT of the scores matmul.
                return kx[64 * h : 64 * h + 64, 128 * kc : 128 * kc + 128]

            def q_slice(h, qs, qe):
                if qe <= 512:
                    return q0[64 * h : 64 * h + 64, qs:qe]
                if qe <= 1024:
                    return q1[64 * h : 64 * h + 64, qs - 512 : qe - 512]
                return q23[64 * h : 64 * h + 64, qs - 1024 : qe - 1024]

            def v_slice(h, kc):
                return vsb[h][:, 65 * kc : 65 * kc + 65]

            def emit_span(s, po, flush_tails):
                qs, qe = 512 * s, 512 * (s + 1)
                # Tiles: non-diag chunks kc<4s get one [128,1024] tile each
                # (h0 block [0:512] bank A, h1 [512:1024] bank B). Diagonal
                # chunks are end-aligned: h0 at [512-w:512], h1 at [512:512+w],
                # so the exp covers 2w contiguous columns with no gap waste.
                # The two smallest diagonal chunks (w=256,128) share one tile.
                tiles = [[kc] for kc in range(4 * s)]
                tiles.append([4 * s])      # diag w=512 (full block, needs mask)
                tiles.append([4 * s + 1])  # diag w=384
                tiles.append([4 * s + 2, 4 * s + 3])  # diag w=256,128 merged
                for tix, kcs in enumerate(tiles):
                    if tix == 1:
                        flush_tails()
                    diag = kcs[0] >= 4 * s
                    pg = ps_s.tile([128, 1024], F32, tag="pss", name=f"pg_{s}_{kcs[0]}")
                    pe2 = pbuf.tile([128, 1024], BF16, tag="pe", name=f"pe_{s}_{kcs[0]}")
                    ws = [qe - max(qs, 128 * kc) for kc in kcs]
                    tot = sum(ws)
                    blocks = []  # (h, kc, off, w)
                    o = 512 - tot
                    for kc, w in reversed(list(zip(kcs, ws))):
                        blocks.append((0, kc, o, w))
                        o += w
                    o = 512
                    for kc, w in zip(kcs, ws):
                        blocks.append((1, kc, o, w))
                        o += w
                    lo, hi = 512 - tot, 512 + tot
                    # Score matmuls: per kc the (h0, h1) pair runs concurrently
                    # on PE row groups 0-63 / 64-127, draining into banks A/B.
                    order = sorted(blocks, key=lambda b: (b[1], b[0]))
                    if s == 0 and tix == 0:
                        # PE prewarm: dummy matmuls that only depend on the
                        # junk memset, emitted into this tile's psum (the real
                        # start=True score matmuls below overwrite it). Keeps
                        # the PE continuously busy through the first DMA wait
                        # so the HAM throttle reaches full speed beforehand.
                        for _ in range(PREWARM_MMS):
                            nc.tensor.matmul(
                                pg[:, 0:128], junk, junk, start=True, stop=True
                            )
                    # Causal masks for diagonal chunks: accumulate the -480
                    # strictly-lower-triangular bias on the PE (weights =
                    # identity, streamed = btri) so exp flushes masked entries
                    # to zero with no DVE involvement. The bias (start=False)
                    # must land before the next start=True matmul in the same
                    # bank (start=True clears has_written bank-wide), so emit
                    # per chunk: (h0, h1) score pair, then (h0, h1) bias pair.
                    for kc, w in zip(kcs, ws):
                        qb = qe - w
                        chunk = [b for b in order if b[1] == kc]
                        for h, _, off, _ in chunk:
                            nc.tensor.matmul(
                                pg[:, off : off + w],
                                k_slice(h, kc),
                                q_slice(h, qb, qe),
                                start=True,
                                stop=not diag,
                            )
                        if diag:
                            for h, _, off, _ in chunk:
                                nc.tensor.matmul(
                                    pg[:, off : off + w],
                                    ident,
                                    btri[:, 0:w],
                                    start=False,
                                    stop=True,
                                )
                    # One exp for the whole tile (both heads, all its chunks).
                    nc.scalar.activation(
                        out=pe2[:, lo:hi],
                        in_=pg[:, lo:hi],
                        func=mybir.ActivationFunctionType.Exp,
                        scale=0.125,
                    )
                    # PV accumulation.
                    nkc = 4 * s + 4
                    for h, kc, off, w in order:
                        qb = qe - w
                        nc.tensor.matmul(
                            po[h][:, qb - qs : qb - qs + w],
                            v_slice(h, kc),
                            pe2[:, off : off + w],
                            start=(kc == 0),
                            stop=(kc == nkc - 1),
                        )

            def emit_tail(s, h, po):
                # 1/denom: copy the psum denominator row to a partition-0 SBUF
                # tile (custom DVE ops mis-read at nonzero partition offsets),
                # fast approx reciprocal (~51 ULP), broadcast across
                # partitions on the otherwise-idle gpsimd, normalize on DVE
                # (PSUM x SBUF -> f16), store via sync (h0) / vector (h1).
                dn = nbuf.tile([1, 512], F32, tag=f"dn{h}", name=f"dn{h}_{s}")
                r = nbuf.tile([1, 512], F32, tag=f"r{h}", name=f"r{h}_{s}")
                rb = nbuf.tile([64, 512], F32, tag=f"rb{h}", name=f"rb{h}_{s}")
                o_sb = nbuf.tile([64, 512], F16, tag=f"o{h}", name=f"o{h}_{s}")
                nc.vector.tensor_copy(out=dn, in_=po[h][64:65, :])
                nc.vector.reciprocal_approx_fast(out=r, in_=dn)
                nc.gpsimd.partition_broadcast(rb[:, :], r[0:1, :])
                nc.vector.tensor_mul(out=o_sb, in0=po[h][0:64, :], in1=rb)
                eng = nc.sync if h == 0 else nc.gpsimd
                eng.dma_start(out=outT[h, :, 512 * s : 512 * (s + 1)], in_=o_sb)

            def emit_final_tail(po):
                # The final span's tails are the only exposed ones: pipeline
                # in column halves with the denominator copies on the now-idle
                # scalar engine, stages interleaved across both heads so the
                # two chains overlap instead of serializing.
                dn, r, rb, o_sb = {}, {}, {}, {}
                for h in range(2):
                    dn[h] = nbuf.tile([1, 512], F32, tag=f"dn{h}", name=f"dnf{h}")
                    r[h] = nbuf.tile([1, 512], F32, tag=f"r{h}", name=f"rf{h}")
                    rb[h] = nbuf.tile([64, 512], F32, tag=f"rb{h}", name=f"rbf{h}")
                    o_sb[h] = nbuf.tile([64, 512], F16, tag=f"o{h}", name=f"of{h}")
                for a, b in ((0, 256), (256, 512)):
                    for h in range(2):
                        nc.scalar.copy(out=dn[h][:, a:b], in_=po[h][64:65, a:b])
                    for h in range(2):
                        nc.vector.reciprocal_approx_fast(
                            out=r[h][:, a:b], in_=dn[h][:, a:b]
                        )
                    for h in range(2):
                        nc.gpsimd.partition_broadcast(rb[h][:, a:b], r[h][0:1, a:b])
                    for h in range(2):
                        nc.vector.tensor_mul(
                            out=o_sb[h][:, a:b],
                            in0=po[h][0:64, a:b],
                            in1=rb[h][:, a:b],
                        )
                    for h in range(2):
                        eng = nc.sync if h == 0 else nc.gpsimd
                        eng.dma_start(
                            out=outT[h, :, 1536 + a : 1536 + b], in_=o_sb[h][:, a:b]
                        )

            pending = []

            def flush_tails():
                while pending:
                    ps, ppo = pending.pop(0)
                    for h in range(2):
                        emit_tail(ps, h, ppo)

            for s in (0, 1, 2, 3):
                po = [
                    ps_o.tile([65, 512], F32, tag=f"po{hh}", name=f"po{hh}_{s}")
                    for hh in range(2)
                ]
                emit_span(s, po, flush_tails)
                pending.append((s, po))
            _, po3 = pending.pop(-1)
            flush_tails()
            emit_final_tail(po3)

    nc.compile()
    return nc


def _get_nc():
    global _NC
    if _NC is None:
        _NC = _build_bass()
    return _NC


def kernel(q, k, v):
    global _LAST_RESULTS
    q = np.asarray(q, dtype=np.float32)
    k = np.asarray(k, dtype=np.float32)
    v = np.asarray(v, dtype=np.float32)
    assert q.shape == (B, H, S, D)

    in_maps = []
    for c in range(N_CORES):
        h0 = HEADS_PER_CORE * c
        qTh = np.ascontiguousarray(
            q[0, h0 : h0 + 2].transpose(0, 2, 1).reshape(128, S)
        ).astype(np.float16)
        kTh = np.ascontiguousarray(
            k[0, h0 : h0 + 2].transpose(0, 2, 1).reshape(128, S)
        ).astype(np.float16)
        va = np.ones((2, 128, N_CHUNKS, 65), dtype=np.float32)
        va[..., :64] = (
            v[0, h0 : h0 + 2].reshape(2, N_CHUNKS, 128, 64).transpose(0, 2, 1, 3)
        )
        va16 = va.reshape(2, 128, 65 * N_CHUNKS).astype(ml_dtypes.bfloat16)
        in_maps.append({"qT": qTh, "kT": kTh, "vaug": va16})

    nc = _get_nc()
    res = run_bass_kernel_spmd(nc, in_maps, core_ids=list(range(N_CORES)))
    _LAST_RESULTS = res

    out = np.empty((B, H, S, D), dtype=np.float32)
    for c in range(N_CORES):
        ot = res.results[c]["outT"].astype(np.float32)  # [2, 64, 2048] f16
        out[0, 2 * c] = ot[0].T
        out[0, 2 * c + 1] = ot[1].T
    return out


# revision 15
# speedup vs baseline: 1.1250x; 1.1250x over previous
"""Causal flash attention for Trainium2, sharded 2 heads/core over 8 cores.

Math per head: out = softmax_causal(Q K^T / sqrt(D)) @ V,  Q/K/V [S=2048, D=64] fp32.

Device layout (per core, heads h0=2c, h1=2c+1):
  qT   [128, 2048]  rows 64h+d = Q[h]^T        (D on partitions, both heads stacked)
  kT   [128, 2048]  same for K
  vaug [2, 128, 1040] vaug[h, p, 65*kc+d] = V[h, 128*kc+p, d], d=64 column is ones
  outT [2, 64, 2048]  out[h]^T (normalized)

Scores are computed transposed (S^T[k, q] = K_chunk @ Q^T) so no transposes are
needed anywhere: softmax denominator comes out of the PV matmul via the ones
column of vaug (psum row 64), and the final division broadcasts 1/denom across
partitions with gpsimd.partition_broadcast.

The kernel is paced by the Scalar (ACT) engine's exp over every causal score
(34816 columns/core at 1 elem/cycle/lane, 1.2 GHz). Structure keeps ACT
saturated at that floor:
  - scores land in psum [128,1024] tiles (h0 in bank A, h1 in bank B; the
    head-pair matmuls run concurrently on PE row groups 0-63/64-127), one exp
    instruction per tile covering both heads.
  - diagonal chunks (width w < 512) are END-aligned against the bank boundary
    (h0 at [512-w:512], h1 at [512:512+w]) so the exp covers 2w contiguous
    columns with zero gap waste; the two smallest diagonal chunks of each span
    share one tile, giving 36 ACTIVATEs and exactly S^2/2-worth of columns.
  - causal masking is folded into the PE: an extra accumulating matmul adds a
    precomputed -480 strictly-lower-triangular bias (exp(0.125*x - 60) == 0)
    into the diagonal psum blocks before the exp, so the DVE carries no mask
    work and the exp->PV chain has no cross-engine mask hop.
  - the PE is prewarmed with dummy matmuls during the initial DMA wait so the
    HAM throttle ramps to full speed (2.4 GHz) before the first real matmul
    instead of ~6us into the kernel.
  - inputs arrive as ten DMAs on the sync/vector/gpsimd queues, smallest and
    first-needed pieces first (q span 0 + k chunk 0 gate the first matmul);
    the Scalar queue carries nothing but the exps.
  - each span's normalization tail is emitted one span late so every tail
    except the last hides under the next span's exp stream; the last tail is
    pipelined in column halves with its denominator copies on the then-idle
    scalar engine and the four stages interleaved across both heads.

HW-verified pitfalls honored here: custom DVE ops (reciprocal_approx_fast)
mis-read operands at nonzero partition offsets and from PSUM (denominator row
is first copied to a partition-0 SBUF tile); matmul PSUM output must be fp32
on TRN2; DVE instructions may read at most one PSUM operand; PE-write and
engine-read of the same PSUM bank is fatal (tails only read po after its
accumulation group closes).
"""

import os
import sys

import ml_dtypes
import numpy as np

sys.path.insert(0, "/opt/trn_rl_repo")

import concourse.bass as bass
import concourse.bacc as bacc
import concourse.mybir as mybir
import concourse.tile as tile
from concourse.bass_utils import run_bass_kernel_spmd

B, H, S, D = 1, 16, 2048, 64
N_CORES = 8
HEADS_PER_CORE = H // N_CORES  # 2
N_CHUNKS = S // 128  # 16 key chunks per head
N_SPANS = S // 512  # 4 query spans per head
F32 = mybir.dt.float32
BF16 = mybir.dt.bfloat16
F16 = mybir.dt.float16

PREWARM_MMS = 24  # dummy matmuls to ramp the PE while the first DMAs land

_NC = None
_LAST_RESULTS = None


def _build_bass():
    nc = bacc.Bacc("TRN2", target_bir_lowering=False)
    qT = nc.declare_dram_parameter("qT", [128, S], F16, isOutput=False)
    kT = nc.declare_dram_parameter("kT", [128, S], F16, isOutput=False)
    vaug = nc.declare_dram_parameter("vaug", [2, 128, 65 * N_CHUNKS], BF16, isOutput=False)
    outT = nc.declare_dram_parameter("outT", [2, 64, S], F16, isOutput=True)

    with tile.TileContext(nc) as tc:
        with (
            tc.tile_pool(name="const", bufs=1) as const,
            tc.tile_pool(name="inbuf", bufs=1) as inbuf,
            tc.tile_pool(name="pbuf", bufs=6) as pbuf,
            tc.tile_pool(name="nbuf", bufs=2) as nbuf,
            tc.tile_pool(name="ps_s", bufs=2, space="PSUM") as ps_s,
            tc.tile_pool(name="ps_o", bufs=2, space="PSUM") as ps_o,
        ):
            # Prewarm source: gpsimd's very first op so the PE can start
            # ramping immediately (before gpsimd turns to its DMA issues).
            junk = const.tile([64, 128], BF16, tag="junk", name="junk")
            nc.gpsimd.memset(junk, 0.0)

            # Input loads. Scalar stays clean (it is the pacing engine); only
            # the sync and gpsimd queues can issue DMAs besides it. Pieces are
            # ordered by first-need time: q0 + k chunks 0-3 gate span 0.
            q0 = inbuf.tile([128, 512], F16, tag="q0", name="q0")
            nc.sync.dma_start(out=q0, in_=qT[:, 0:512])
            v0 = inbuf.tile([128, 65 * N_CHUNKS], BF16, tag="v0", name="v0")
            nc.sync.dma_start(out=v0[:, 0:260], in_=vaug[0][:, 0:260])
            q1 = inbuf.tile([128, 512], F16, tag="q1", name="q1")
            nc.sync.dma_start(out=q1, in_=qT[:, 512:1024])
            nc.sync.dma_start(out=v0[:, 260:1040], in_=vaug[0][:, 260:1040])
            q23 = inbuf.tile([128, 1024], F16, tag="q23", name="q23")
            nc.sync.dma_start(out=q23, in_=qT[:, 1024:2048])

            kx = inbuf.tile([128, 2048], F16, tag="kx", name="kx")
            nc.gpsimd.dma_start(out=kx[:, 0:512], in_=kT[:, 0:512])
            v1 = inbuf.tile([128, 65 * N_CHUNKS], BF16, tag="v1", name="v1")
            nc.gpsimd.dma_start(out=v1[:, 0:260], in_=vaug[1][:, 0:260])
            nc.gpsimd.dma_start(out=kx[:, 512:1024], in_=kT[:, 512:1024])
            nc.gpsimd.dma_start(out=v1[:, 260:1040], in_=vaug[1][:, 260:1040])
            nc.gpsimd.dma_start(out=kx[:, 1024:2048], in_=kT[:, 1024:2048])
            vsb = [v0, v1]

            # Diagonal-mask constant (multiply-mask applied on the DVE after
            # the exp, off the scalar engine's critical path) and the ones
            # row used by the final tail's PE partition-broadcast.
            mtri = const.tile([128, 128], BF16, tag="mtri", name="mtri")
            nc.gpsimd.memset(mtri, 1.0)
            nc.gpsimd.affine_select(
                out=mtri,
                in_=mtri,
                compare_op=mybir.AluOpType.is_ge,
                fill=0.0,
                base=0,
                pattern=[[1, 128]],
                channel_multiplier=-1,
            )
            ones64 = const.tile([1, 64], BF16, tag="ones64", name="ones64")
            nc.gpsimd.memset(ones64, 1.0)

            def k_slice(h, kc):
                # kT chunk [64, 128] for head h: lhsT of the scores matmul.
                return kx[64 * h : 64 * h + 64, 128 * kc : 128 * kc + 128]

            def q_slice(h, qs, qe):
                if qe <= 512:
                    return q0[64 * h : 64 * h + 64, qs:qe]
                if qe <= 1024:
                    return q1[64 * h : 64 * h + 64, qs - 512 : qe - 512]
                return q23[64 * h : 64 * h + 64, qs - 1024 : qe - 1024]

            def v_slice(h, kc):
                return vsb[h][:, 65 * kc : 65 * kc + 65]

            def emit_span(s, po, flush_tails):
                qs, qe = 512 * s, 512 * (s + 1)
                # Tiles: non-diag chunks kc<4s get one [128,1024] tile each
                # (h0 block [0:512] bank A, h1 [512:1024] bank B). Diagonal
                # chunks are end-aligned: h0 at [512-w:512], h1 at [512:512+w],
                # so the exp covers 2w contiguous columns with no gap waste.
                # The two smallest diagonal chunks (w=256,128) share one tile.
                tiles = [[kc] for kc in range(4 * s)]
                tiles.append([4 * s])      # diag w=512 (full block, needs mask)
                tiles.append([4 * s + 1])  # diag w=384
                tiles.append([4 * s + 2, 4 * s + 3])  # diag w=256,128 merged
                for tix, kcs in enumerate(tiles):
                    if tix == 1:
                        flush_tails()
                    diag = kcs[0] >= 4 * s
                    pg = ps_s.tile([128, 1024], F32, tag="pss", name=f"pg_{s}_{kcs[0]}")
                    pe2 = pbuf.tile([128, 1024], BF16, tag="pe", name=f"pe_{s}_{kcs[0]}")
                    ws = [qe - max(qs, 128 * kc) for kc in kcs]
                    tot = sum(ws)
                    blocks = []  # (h, kc, off, w)
                    o = 512 - tot
                    for kc, w in reversed(list(zip(kcs, ws))):
                        blocks.append((0, kc, o, w))
                        o += w
                    o = 512
                    for kc, w in zip(kcs, ws):
                        blocks.append((1, kc, o, w))
                        o += w
                    lo, hi = 512 - tot, 512 + tot
                    # Score matmuls: per kc the (h0, h1) pair runs concurrently
                    # on PE row groups 0-63 / 64-127, draining into banks A/B.
                    order = sorted(blocks, key=lambda b: (b[1], b[0]))
                    if s == 0 and tix == 0:
                        # PE prewarm: dummy matmuls that only depend on the
                        # junk memset, emitted into this tile's psum (the real
                        # start=True score matmuls below overwrite it). Keeps
                        # the PE continuously busy through the first DMA wait
                        # so the HAM throttle reaches full speed beforehand.
                        for _ in range(PREWARM_MMS):
                            nc.tensor.matmul(
                                pg[:, 0:128], junk, junk, start=True, stop=True
                            )
                    for h, kc, off, w in order:
                        qb = qe - w
                        nc.tensor.matmul(
                            pg[:, off : off + w],
                            k_slice(h, kc),
                            q_slice(h, qb, qe),
                            start=True,
                            stop=True,
                        )
                    # One exp for the whole tile (both heads, all its chunks).
                    nc.scalar.activation(
                        out=pe2[:, lo:hi],
                        in_=pg[:, lo:hi],
                        func=mybir.ActivationFunctionType.Exp,
                        scale=0.125,
                    )
                    # Causal masks for diagonal chunks: zero the strict upper
                    # triangle of each block's first 128 query columns (DVE).
                    if diag:
                        for h, kc, off, w in order:
                            nc.vector.tensor_mul(
                                out=pe2[:, off : off + 128],
                                in0=pe2[:, off : off + 128],
                                in1=mtri,
                            )
                    # PV accumulation.
                    nkc = 4 * s + 4
                    for h, kc, off, w in order:
                        qb = qe - w
                        nc.tensor.matmul(
                            po[h][:, qb - qs : qb - qs + w],
                            v_slice(h, kc),
                            pe2[:, off : off + w],
                            start=(kc == 0),
                            stop=(kc == nkc - 1),
                        )

            def emit_tail(s, h, po):
                # 1/denom: copy the psum denominator row to a partition-0 SBUF
                # tile (custom DVE ops mis-read at nonzero partition offsets),
                # fast approx reciprocal (~51 ULP), broadcast across
                # partitions on the otherwise-idle gpsimd, normalize on DVE
                # (PSUM x SBUF -> f16), store via sync (h0) / vector (h1).
                dn = nbuf.tile([1, 512], F32, tag=f"dn{h}", name=f"dn{h}_{s}")
                r = nbuf.tile([1, 512], F32, tag=f"r{h}", name=f"r{h}_{s}")
                rb = nbuf.tile([64, 512], F32, tag=f"rb{h}", name=f"rb{h}_{s}")
                o_sb = nbuf.tile([64, 512], F16, tag=f"o{h}", name=f"o{h}_{s}")
                nc.vector.tensor_copy(out=dn, in_=po[h][64:65, :])
                nc.vector.reciprocal_approx_fast(out=r, in_=dn)
                nc.gpsimd.partition_broadcast(rb[:, :], r[0:1, :])
                nc.vector.tensor_mul(out=o_sb, in0=po[h][0:64, :], in1=rb)
                nc.sync.dma_start(out=outT[h, :, 512 * s : 512 * (s + 1)], in_=o_sb)

            def emit_final_tail(po):
                # The final span's tails are the only exposed ones. Shortest
                # chain per head: denominator row -> SBUF on the now-idle
                # scalar engine, bf16 reciprocal on DVE, partition-broadcast
                # via a rank-1 PE matmul (ones64^T @ r, ~0.2us vs gpsimd's
                # 0.64us custom op) into a retired po-pool psum bank, copy the
                # broadcast to SBUF on scalar, then one DVE mul (PSUM x SBUF)
                # and the store. Both heads' chains interleave per stage.
                dn, rp, rsb, rr, o_sb = {}, {}, {}, {}, {}
                for h in range(2):
                    dn[h] = nbuf.tile([1, 512], BF16, tag=f"dn{h}", name=f"dnf{h}")
                    rp[h] = ps_o.tile([65, 512], F32, tag=f"po{h}", name=f"rp{h}")
                    rsb[h] = nbuf.tile([64, 512], F32, tag=f"rb{h}", name=f"rsb{h}")
                    rr[h] = nbuf.tile([64, 512], F32, tag=f"rr{h}", name=f"rr{h}")
                    o_sb[h] = nbuf.tile([64, 512], F16, tag=f"o{h}", name=f"of{h}")
                for h in range(2):
                    nc.scalar.copy(out=dn[h], in_=po[h][64:65, :])
                for h in range(2):
                    nc.tensor.matmul(
                        rp[h][0:64, :], ones64, dn[h], start=True, stop=True
                    )
                for h in range(2):
                    nc.scalar.copy(out=rsb[h], in_=rp[h][0:64, :])
                for h in range(2):
                    nc.vector.reciprocal_approx_fast(out=rr[h], in_=rsb[h])
                for h in range(2):
                    nc.vector.tensor_mul(
                        out=o_sb[h], in0=po[h][0:64, :], in1=rr[h]
                    )
                    nc.sync.dma_start(out=outT[h, :, 1536:2048], in_=o_sb[h])

            pending = []

            def flush_tails():
                while pending:
                    ps, ppo = pending.pop(0)
                    for h in range(2):
                        emit_tail(ps, h, ppo)

            for s in (0, 1, 2, 3):
                po = [
                    ps_o.tile([65, 512], F32, tag=f"po{hh}", name=f"po{hh}_{s}")
                    for hh in range(2)
                ]
                emit_span(s, po, flush_tails)
                pending.append((s, po))
            _, po3 = pending.pop(-1)
            flush_tails()
            emit_final_tail(po3)

    nc.compile()
    return nc


def _get_nc():
    global _NC
    if _NC is None:
        _NC = _build_bass()
    return _NC


def kernel(q, k, v):
    global _LAST_RESULTS
    q = np.asarray(q, dtype=np.float32)
    k = np.asarray(k, dtype=np.float32)
    v = np.asarray(v, dtype=np.float32)
    assert q.shape == (B, H, S, D)

    in_maps = []
    for c in range(N_CORES):
        h0 = HEADS_PER_CORE * c
        qTh = np.ascontiguousarray(
            q[0, h0 : h0 + 2].transpose(0, 2, 1).reshape(128, S)
        ).astype(np.float16)
        kTh = np.ascontiguousarray(
            k[0, h0 : h0 + 2].transpose(0, 2, 1).reshape(128, S)
        ).astype(np.float16)
        va = np.ones((2, 128, N_CHUNKS, 65), dtype=np.float32)
        va[..., :64] = (
            v[0, h0 : h0 + 2].reshape(2, N_CHUNKS, 128, 64).transpose(0, 2, 1, 3)
        )
        va16 = va.reshape(2, 128, 65 * N_CHUNKS).astype(ml_dtypes.bfloat16)
        in_maps.append({"qT": qTh, "kT": kTh, "vaug": va16})

    nc = _get_nc()
    res = run_bass_kernel_spmd(nc, in_maps, core_ids=list(range(N_CORES)))
    _LAST_RESULTS = res

    out = np.empty((B, H, S, D), dtype=np.float32)
    for c in range(N_CORES):
        ot = res.results[c]["outT"].astype(np.float32)  # [2, 64, 2048] f16
        out[0, 2 * c] = ot[0].T
        out[0, 2 * c + 1] = ot[1].T
    return out
